# revision 32
# baseline (speedup 1.0000x reference)
"""Additive attention (Bahdanau-style) Trainium2 kernel, 8-core data-parallel.

Reference computation (per batch b):
    proj_hidden[b, :]  = hidden_state[b] @ W_hidden.T                  # [A]
    combined[b, s, :]  = tanh(proj_image[b, s] + proj_hidden[b])       # [S, A]
    scores[b, s]       = combined[b, s] @ w_score                      # [S]
    weights[b, :]      = softmax(scores[b])                            # [S]
    context[b, :]      = weights[b] @ image_features[b]                # [FDIM]

Sharding: B=64 split as 8 batches per core; W_hidden / w_score replicated.

Per-core layout: S is tiled as s = j*128 + p with p on SBUF partitions.
  - proj tile  [128(p), 32(j)*128(a)]   (2 MiB, one DMA)
  - feat tile  [128(p), 32(j)*512(d)]   (8 MiB, one DMA)
  - scores     [128(p), 32(j)]  -> softmax entirely along free+partition axes
  - context    accumulated on TensorE: sum_j weights[:, j].T @ feat[:, j, :]
Softmax skips the max-subtraction: |scores| <= sum|w_score| ~ 9, safe in f32.
"""

import numpy as np
from contextlib import ExitStack

B, S, A, FDIM, HDIM = 64, 4096, 128, 512, 512
NCORES = 8
BL = B // NCORES  # 8 batches per core
P = 128
NJ = S // P  # 32
NK = HDIM // P  # 4
NCH = 4  # feat DMA chunks per batch
JC = NJ // NCH  # j's per feat chunk


def _build_nc():
    import concourse.bass as bass
    import concourse.tile as tile
    from concourse import bacc, mybir, bass_isa
    from concourse.masks import make_identity

    dt = mybir.dt.float32
    bf = mybir.dt.bfloat16
    AF = mybir.ActivationFunctionType

    nc = bacc.Bacc(
        "TRN2",
        target_bir_lowering=False,
        debug=False,
        enable_asserts=True,
        num_devices=NCORES,
    )

    f32r = mybir.dt.float32r
    proj = nc.dram_tensor("proj_image", [BL, S, A], dt, kind="ExternalInput").ap()
    # float32r: identical f32 bits on the IO path (dt.np(float32r) == float32),
    # but lets the context matmul run in fp32r mode (full-rate streaming vs 1/4
    # for plain fp32) with the BIR verifier's producer-dtype rule satisfied.
    feat = nc.dram_tensor(
        "image_features", [BL, S, FDIM], f32r, kind="ExternalInput"
    ).ap()
    hs = nc.dram_tensor("hidden_state", [BL, HDIM], dt, kind="ExternalInput").ap()
    W = nc.dram_tensor("W_hidden", [A, HDIM], dt, kind="ExternalInput").ap()
    w = nc.dram_tensor("w_score", [A], dt, kind="ExternalInput").ap()
    ctx_out = nc.dram_tensor("context", [BL, FDIM], dt, kind="ExternalOutput").ap()
    wts_out = nc.dram_tensor("weights", [BL, S], dt, kind="ExternalOutput").ap()

    with tile.TileContext(nc) as tc, ExitStack() as ctx:
        const = ctx.enter_context(tc.tile_pool(name="const", bufs=1))

        ident = const.tile([P, P], dt)
        make_identity(nc, ident[:])

        # w_score broadcast to all partitions: w_bc[p, a] = w_score[a]
        w_bc = const.tile([P, A], dt)
        nc.sync.dma_start(w_bc[:], w[None, :].broadcast_to([P, A]))

        # proj_hidden = hs @ W.T  (contract over h: transpose both onto h-partitions)
        W_sb = const.tile([A, HDIM], dt)
        nc.sync.dma_start(W_sb[:], W[:, :])
        hs_sb = const.tile([BL, HDIM], dt)
        nc.sync.dma_start(hs_sb[:], hs[:, :])

        WT = const.tile([P, NK * A], dt)  # chunk k: WT[h, a] = W[a, k*128+h]
        hsT = const.tile([P, NK * BL], dt)  # chunk k: hsT[h, b] = hs[b, k*128+h]
        ph_sb = const.tile([BL, A], dt)
        with tc.tile_pool(name="psum_setup", bufs=2, space="PSUM") as psum:
            for k in range(NK):
                pt = psum.tile([P, A], dt, tag="tp")
                nc.tensor.transpose(pt[:], W_sb[:, k * P:(k + 1) * P], ident[:])
                nc.vector.tensor_copy(WT[:, k * A:(k + 1) * A], pt[:])
                pt2 = psum.tile([P, BL], dt, tag="tp2")
                nc.tensor.transpose(
                    pt2[:], hs_sb[:, k * P:(k + 1) * P], ident[:BL, :BL]
                )
                nc.vector.tensor_copy(hsT[:, k * BL:(k + 1) * BL], pt2[:])

            ph_psum = psum.tile([BL, A], dt, tag="ph")
            for k in range(NK):
                nc.tensor.matmul(
                    ph_psum[:],
                    lhsT=hsT[:, k * BL:(k + 1) * BL],
                    rhs=WT[:, k * A:(k + 1) * A],
                    start=(k == 0),
                    stop=(k == NK - 1),
                )
            nc.vector.tensor_copy(ph_sb[:], ph_psum[:])

        # ph_bc[b]: proj_hidden[b, :] broadcast to all 128 partitions.
        # Engines can't broadcast across partitions from an arbitrary start
        # partition, so roundtrip through DRAM and use a step-0 DMA source AP.
        ph_dram = nc.dram_tensor("ph_scratch", [BL, A], dt).ap()
        nc.sync.dma_start(ph_dram[:, :], ph_sb[:])
        ph_bc = const.tile([P, BL * A], dt)
        nc.sync.dma_start(
            ph_bc[:].rearrange("p (b a) -> p b a", a=A),
            ph_dram[None, :, :].broadcast_to([P, BL, A]),
        )

        proj_pool = ctx.enter_context(tc.tile_pool(name="proj", bufs=2))
        feat_pool = ctx.enter_context(tc.tile_pool(name="feat", bufs=2))
        sc_pool = ctx.enter_context(tc.tile_pool(name="sc", bufs=2))
        out_pool = ctx.enter_context(tc.tile_pool(name="outs", bufs=1))
        cpsum = ctx.enter_context(tc.tile_pool(name="cpsum", bufs=2, space="PSUM"))

        ctx_all = out_pool.tile([1, BL * FDIM], dt)  # context rows, concatenated

        # s is tiled as s = p*NJ + j: each partition reads ONE contiguous run
        # per DMA (16KB proj / 64KB feat) -> 128 big descriptors instead of
        # 4096 small ones, so HWDGE descriptor generation can't throttle DMA.
        for b in range(BL):
            pt = proj_pool.tile([P, S], dt, tag="proj")
            p3 = pt[:].rearrange("p (j a) -> p j a", a=A)
            nc.sync.dma_start(
                pt[:], proj[b].rearrange("s a -> (s a)").rearrange("(p x) -> p x", p=P)
            )
            # One 8 MiB DMA per batch: 64KB contiguous per partition is the
            # descriptor size that sustains line rate (smaller chunks measured
            # strictly slower end-to-end).
            ft = feat_pool.tile([P, NJ * FDIM], f32r, tag="feat")
            f3 = ft[:].rearrange("p (j d) -> p j d", d=FDIM)
            nc.sync.dma_start(
                ft[:], feat[b].rearrange("s d -> (s d)").rearrange("(p x) -> p x", p=P)
            )
            ph_b = ph_bc[:, b * A:(b + 1) * A][:, None, :].broadcast_to([P, NJ, A])
            w_b = w_bc[:][:, None, :].broadcast_to([P, NJ, A])

            # in-place: proj += ph ; proj = tanh(proj) ; proj *= w ; reduce over a
            nc.vector.tensor_add(p3, p3, ph_b)
            nc.scalar.activation(pt[:], pt[:], AF.Tanh)
            nc.vector.tensor_mul(p3, p3, w_b)
            scores = sc_pool.tile([P, NJ], dt, tag="scores")
            nc.vector.tensor_reduce(
                scores[:], p3, axis=mybir.AxisListType.X, op=mybir.AluOpType.add
            )

            # softmax over all 4096 (partitions x 32 cols), no max-subtraction
            rowsum = sc_pool.tile([P, 1], dt, tag="rowsum")
            nc.scalar.activation(scores[:], scores[:], AF.Exp, accum_out=rowsum[:])
            tot = sc_pool.tile([P, 1], dt, tag="tot")
            nc.gpsimd.partition_all_reduce(
                tot[:], rowsum[:], channels=P, reduce_op=bass_isa.ReduceOp.add
            )
            nc.vector.reciprocal(tot[:], tot[:])
            nc.vector.tensor_scalar_mul(scores[:], scores[:], tot[:])

            # weights out: with s = p*NJ + j, scores[p, j] maps straight onto
            # DRAM rows -- write directly on the idle SWDGE (gpsimd) queue.
            nc.gpsimd.dma_start(
                wts_out[b].rearrange("(p j) -> p j", p=P), scores[:]
            )

            # context: accumulate sum_j weights[:, j].T @ feat[:, j, :] on TensorE.
            # float32r streams at full rate for moving dim >= 256 (f32 is 1/4).
            scores_r = sc_pool.tile([P, NJ], f32r, tag="scores_r")
            nc.vector.tensor_copy(scores_r[:], scores[:])
            cp = cpsum.tile([1, FDIM], dt, tag="ctx")
            for j in range(NJ):
                nc.tensor.matmul(
                    cp[:],
                    lhsT=scores_r[:, j:j + 1],
                    rhs=f3[:, j, :],
                    start=(j == 0),
                    stop=(j == NJ - 1),
                )
            nc.vector.tensor_copy(ctx_all[:, b * FDIM:(b + 1) * FDIM], cp[:])

        nc.gpsimd.dma_start(ctx_out.rearrange("b d -> (b d)")[None, :], ctx_all[:])

    nc.compile()
    return nc


_NC_CACHE = None


def _get_nc():
    global _NC_CACHE
    if _NC_CACHE is None:
        _NC_CACHE = _build_nc()
    return _NC_CACHE


def kernel(proj_image, image_features, hidden_state, W_hidden, w_score):
    from concourse.bass_utils import run_bass_kernel_spmd

    proj_image = np.ascontiguousarray(np.asarray(proj_image, dtype=np.float32))
    image_features = np.ascontiguousarray(np.asarray(image_features, dtype=np.float32))
    hidden_state = np.ascontiguousarray(np.asarray(hidden_state, dtype=np.float32))
    W_hidden = np.ascontiguousarray(np.asarray(W_hidden, dtype=np.float32))
    w_score = np.ascontiguousarray(np.asarray(w_score, dtype=np.float32))

    nc = _get_nc()
    in_maps = []
    for c in range(NCORES):
        lo, hi = c * BL, (c + 1) * BL
        in_maps.append(
            {
                "proj_image": proj_image[lo:hi],
                "image_features": image_features[lo:hi],
                "hidden_state": hidden_state[lo:hi],
                "W_hidden": W_hidden,
                "w_score": w_score,
            }
        )

    res = run_bass_kernel_spmd(nc, in_maps, core_ids=list(range(NCORES)))
    context = np.concatenate([res.results[c]["context"] for c in range(NCORES)], axis=0)
    weights = np.concatenate([res.results[c]["weights"] for c in range(NCORES)], axis=0)
    return context, weights


# revision 34
# speedup vs baseline: 1.0512x; 1.0512x over previous
"""Additive attention (Bahdanau-style) Trainium2 kernel, 8-core data-parallel.

Reference computation (per batch b):
    proj_hidden[b, :]  = hidden_state[b] @ W_hidden.T                  # [A]
    combined[b, s, :]  = tanh(proj_image[b, s] + proj_hidden[b])       # [S, A]
    scores[b, s]       = combined[b, s] @ w_score                      # [S]
    weights[b, :]      = softmax(scores[b])                            # [S]
    context[b, :]      = weights[b] @ image_features[b]                # [FDIM]

Sharding: B=64 split as 8 batches per core; W_hidden / w_score replicated.

Per-core layout: S is tiled as s = p*32 + j with p on SBUF partitions, so each
partition's slice of a batch is ONE contiguous DRAM run (16 KiB proj / 64 KiB
feat) -> 128 large DMA descriptors per transfer, which is what sustains
~400 GB/s; the j*128+p layout (512 B/2 KiB descriptors) measures ~25% slower.
  - proj tile  [128(p), 32(j)*128(a)]   (2 MiB, one DMA per batch)
  - feat tile  [128(p), 32(j)*512(d)]   (8 MiB, one DMA per batch, float32r)
  - scores     [128(p), 32(j)]  -> softmax along free axis + partition_all_reduce
  - context    accumulated on TensorE: sum_j weights[:, j].T @ feat[:, j, :]
Softmax skips the max-subtraction: |scores| <= sum|w_score| ~ 9, safe in f32.
"""

import numpy as np
from contextlib import ExitStack

B, S, A, FDIM, HDIM = 64, 4096, 128, 512, 512
NCORES = 8
BL = B // NCORES  # 8 batches per core
P = 128
NJ = S // P  # 32
NK = HDIM // P  # 4


def _build_nc():
    import concourse.tile as tile
    from concourse import bacc, mybir, bass_isa
    from concourse.masks import make_identity

    dt = mybir.dt.float32
    AF = mybir.ActivationFunctionType

    nc = bacc.Bacc(
        "TRN2",
        target_bir_lowering=False,
        debug=False,
        enable_asserts=True,
        num_devices=NCORES,
    )

    f32r = mybir.dt.float32r
    proj = nc.dram_tensor("proj_image", [BL, S, A], dt, kind="ExternalInput").ap()
    # float32r: identical f32 bits on the IO path (dt.np(float32r) == float32),
    # but lets the context matmul run in fp32r mode (full-rate streaming vs 1/4
    # for plain fp32) with the BIR verifier's producer-dtype rule satisfied.
    feat = nc.dram_tensor(
        "image_features", [BL, S, FDIM], f32r, kind="ExternalInput"
    ).ap()
    hs = nc.dram_tensor("hidden_state", [BL, HDIM], dt, kind="ExternalInput").ap()
    W = nc.dram_tensor("W_hidden", [A, HDIM], dt, kind="ExternalInput").ap()
    w = nc.dram_tensor("w_score", [A], dt, kind="ExternalInput").ap()
    ctx_out = nc.dram_tensor("context", [BL, FDIM], dt, kind="ExternalOutput").ap()
    wts_out = nc.dram_tensor("weights", [BL, S], dt, kind="ExternalOutput").ap()

    with tile.TileContext(nc) as tc, ExitStack() as ctx:
        const = ctx.enter_context(tc.tile_pool(name="const", bufs=1))

        ident = const.tile([P, P], dt)
        make_identity(nc, ident[:])

        # w_score broadcast to all partitions: w_bc[p, a] = w_score[a]
        w_bc = const.tile([P, A], dt)
        nc.sync.dma_start(w_bc[:], w[None, :].broadcast_to([P, A]))

        # proj_hidden = hs @ W.T  (contract over h: transpose both onto h-partitions)
        W_sb = const.tile([A, HDIM], dt)
        nc.sync.dma_start(W_sb[:], W[:, :])
        hs_sb = const.tile([BL, HDIM], dt)
        nc.sync.dma_start(hs_sb[:], hs[:, :])

        WT = const.tile([P, NK * A], dt)  # chunk k: WT[h, a] = W[a, k*128+h]
        hsT = const.tile([P, NK * BL], dt)  # chunk k: hsT[h, b] = hs[b, k*128+h]
        ph_sb = const.tile([BL, A], dt)
        with tc.tile_pool(name="psum_setup", bufs=2, space="PSUM") as psum:
            for k in range(NK):
                pt = psum.tile([P, A], dt, tag="tp")
                nc.tensor.transpose(pt[:], W_sb[:, k * P:(k + 1) * P], ident[:])
                nc.vector.tensor_copy(WT[:, k * A:(k + 1) * A], pt[:])
                pt2 = psum.tile([P, BL], dt, tag="tp2")
                nc.tensor.transpose(
                    pt2[:], hs_sb[:, k * P:(k + 1) * P], ident[:BL, :BL]
                )
                nc.vector.tensor_copy(hsT[:, k * BL:(k + 1) * BL], pt2[:])

            ph_psum = psum.tile([BL, A], dt, tag="ph")
            for k in range(NK):
                nc.tensor.matmul(
                    ph_psum[:],
                    lhsT=hsT[:, k * BL:(k + 1) * BL],
                    rhs=WT[:, k * A:(k + 1) * A],
                    start=(k == 0),
                    stop=(k == NK - 1),
                )
            nc.vector.tensor_copy(ph_sb[:], ph_psum[:])

        # ph_bc[b]: proj_hidden[b, :] broadcast to all 128 partitions.
        # Engines can't broadcast across partitions from an arbitrary start
        # partition, so roundtrip through DRAM and use a step-0 DMA source AP.
        ph_dram = nc.dram_tensor("ph_scratch", [BL, A], dt).ap()
        nc.sync.dma_start(ph_dram[:, :], ph_sb[:])
        ph_bc = const.tile([P, BL * A], dt)
        nc.sync.dma_start(
            ph_bc[:].rearrange("p (b a) -> p b a", a=A),
            ph_dram[None, :, :].broadcast_to([P, BL, A]),
        )

        proj_pool = ctx.enter_context(tc.tile_pool(name="proj", bufs=2))
        feat_pool = ctx.enter_context(tc.tile_pool(name="feat", bufs=2))
        sc_pool = ctx.enter_context(tc.tile_pool(name="sc", bufs=2))
        out_pool = ctx.enter_context(tc.tile_pool(name="outs", bufs=1))
        cpsum = ctx.enter_context(tc.tile_pool(name="cpsum", bufs=2, space="PSUM"))

        ctx_all = out_pool.tile([1, BL * FDIM], dt)  # context rows, concatenated

        # s is tiled as s = p*NJ + j: each partition reads ONE contiguous run
        # per DMA (16KB proj / 64KB feat) -> 128 big descriptors instead of
        # 4096 small ones, so HWDGE descriptor generation can't throttle DMA.
        for b in range(BL):
            pt = proj_pool.tile([P, S], dt, tag="proj")
            p3 = pt[:].rearrange("p (j a) -> p j a", a=A)
            nc.sync.dma_start(
                pt[:], proj[b].rearrange("s a -> (s a)").rearrange("(p x) -> p x", p=P)
            )
            # One 8 MiB DMA per batch: 64KB contiguous per partition is the
            # descriptor size that sustains line rate (smaller chunks measured
            # strictly slower end-to-end).
            ft = feat_pool.tile([P, NJ * FDIM], f32r, tag="feat")
            f3 = ft[:].rearrange("p (j d) -> p j d", d=FDIM)
            nc.sync.dma_start(
                ft[:], feat[b].rearrange("s d -> (s d)").rearrange("(p x) -> p x", p=P)
            )
            ph_b = ph_bc[:, b * A:(b + 1) * A][:, None, :].broadcast_to([P, NJ, A])
            w_b = w_bc[:][:, None, :].broadcast_to([P, NJ, A])

            # in-place: proj += ph ; proj = tanh(proj) ; proj *= w ; reduce over a
            nc.vector.tensor_add(p3, p3, ph_b)
            nc.scalar.activation(pt[:], pt[:], AF.Tanh)
            nc.vector.tensor_mul(p3, p3, w_b)
            scores = sc_pool.tile([P, NJ], dt, tag="scores")
            nc.vector.tensor_reduce(
                scores[:], p3, axis=mybir.AxisListType.X, op=mybir.AluOpType.add
            )

            # softmax over all 4096 (partitions x 32 cols), no max-subtraction
            rowsum = sc_pool.tile([P, 1], dt, tag="rowsum")
            nc.scalar.activation(scores[:], scores[:], AF.Exp, accum_out=rowsum[:])
            tot = sc_pool.tile([P, 1], dt, tag="tot")
            nc.gpsimd.partition_all_reduce(
                tot[:], rowsum[:], channels=P, reduce_op=bass_isa.ReduceOp.add
            )
            nc.vector.reciprocal(tot[:], tot[:])
            nc.vector.tensor_scalar_mul(scores[:], scores[:], tot[:])

            # weights out: with s = p*NJ + j, scores[p, j] maps straight onto
            # DRAM rows -- write directly on the idle SWDGE (gpsimd) queue.
            nc.gpsimd.dma_start(
                wts_out[b].rearrange("(p j) -> p j", p=P), scores[:]
            )

            # context: accumulate sum_j weights[:, j].T @ feat[:, j, :] on TensorE.
            # float32r streams at full rate for moving dim >= 256 (f32 is 1/4).
            scores_r = sc_pool.tile([P, NJ], f32r, tag="scores_r")
            nc.vector.tensor_copy(scores_r[:], scores[:])
            cp = cpsum.tile([1, FDIM], dt, tag="ctx")
            for j in range(NJ):
                nc.tensor.matmul(
                    cp[:],
                    lhsT=scores_r[:, j:j + 1],
                    rhs=f3[:, j, :],
                    start=(j == 0),
                    stop=(j == NJ - 1),
                )
            nc.vector.tensor_copy(ctx_all[:, b * FDIM:(b + 1) * FDIM], cp[:])

        nc.gpsimd.dma_start(ctx_out.rearrange("b d -> (b d)")[None, :], ctx_all[:])

    nc.compile()
    return nc


_NC_CACHE = None


def _get_nc():
    global _NC_CACHE
    if _NC_CACHE is None:
        _NC_CACHE = _build_nc()
    return _NC_CACHE


def kernel(proj_image, image_features, hidden_state, W_hidden, w_score):
    from concourse.bass_utils import run_bass_kernel_spmd

    proj_image = np.ascontiguousarray(np.asarray(proj_image, dtype=np.float32))
    image_features = np.ascontiguousarray(np.asarray(image_features, dtype=np.float32))
    hidden_state = np.ascontiguousarray(np.asarray(hidden_state, dtype=np.float32))
    W_hidden = np.ascontiguousarray(np.asarray(W_hidden, dtype=np.float32))
    w_score = np.ascontiguousarray(np.asarray(w_score, dtype=np.float32))

    nc = _get_nc()
    in_maps = []
    for c in range(NCORES):
        lo, hi = c * BL, (c + 1) * BL
        in_maps.append(
            {
                "proj_image": proj_image[lo:hi],
                "image_features": image_features[lo:hi],
                "hidden_state": hidden_state[lo:hi],
                "W_hidden": W_hidden,
                "w_score": w_score,
            }
        )

    res = run_bass_kernel_spmd(nc, in_maps, core_ids=list(range(NCORES)))
    context = np.concatenate([res.results[c]["context"] for c in range(NCORES)], axis=0)
    weights = np.concatenate([res.results[c]["weights"] for c in range(NCORES)], axis=0)
    return context, weights


# revision 35
# speedup vs baseline: 1.0618x; 1.0101x over previous
"""Additive attention (Bahdanau-style) Trainium2 kernel, 8-core data-parallel.

Reference computation (per batch b):
    proj_hidden[b, :]  = hidden_state[b] @ W_hidden.T                  # [A]
    combined[b, s, :]  = tanh(proj_image[b, s] + proj_hidden[b])       # [S, A]
    scores[b, s]       = combined[b, s] @ w_score                      # [S]
    weights[b, :]      = softmax(scores[b])                            # [S]
    context[b, :]      = weights[b] @ image_features[b]                # [FDIM]

Sharding: B=64 split as 8 batches per core; W_hidden / w_score replicated.

Per-core layout: S is tiled as s = p*32 + j with p on SBUF partitions, so each
partition's slice of a batch is ONE contiguous DRAM run (16 KiB proj / 64 KiB
feat) -> 128 large DMA descriptors per transfer, which is what sustains
~400 GB/s; the j*128+p layout (512 B/2 KiB descriptors) measures ~25% slower.
  - proj tile  [128(p), 32(j)*128(a)]   (2 MiB, one DMA per batch)
  - feat tile  [128(p), 32(j)*512(d)]   (8 MiB, one DMA per batch, float32r)
  - scores     [128(p), 32(j)]  -> softmax along free axis + partition_all_reduce
  - context    accumulated on TensorE: sum_j weights[:, j].T @ feat[:, j, :]
Softmax skips the max-subtraction: |scores| <= sum|w_score| ~ 9, safe in f32.
"""

import numpy as np
from contextlib import ExitStack

B, S, A, FDIM, HDIM = 64, 4096, 128, 512, 512
NCORES = 8
BL = B // NCORES  # 8 batches per core
P = 128
NJ = S // P  # 32
NK = HDIM // P  # 4


def _build_nc():
    import concourse.tile as tile
    from concourse import bacc, mybir, bass_isa
    from concourse.masks import make_identity

    dt = mybir.dt.float32
    AF = mybir.ActivationFunctionType

    nc = bacc.Bacc(
        "TRN2",
        target_bir_lowering=False,
        debug=False,
        enable_asserts=True,
        num_devices=NCORES,
    )

    f32r = mybir.dt.float32r
    proj = nc.dram_tensor("proj_image", [BL, S, A], dt, kind="ExternalInput").ap()
    # float32r: identical f32 bits on the IO path (dt.np(float32r) == float32),
    # but lets the context matmul run in fp32r mode (full-rate streaming vs 1/4
    # for plain fp32) with the BIR verifier's producer-dtype rule satisfied.
    feat = nc.dram_tensor(
        "image_features", [BL, S, FDIM], f32r, kind="ExternalInput"
    ).ap()
    hs = nc.dram_tensor("hidden_state", [BL, HDIM], dt, kind="ExternalInput").ap()
    W = nc.dram_tensor("W_hidden", [A, HDIM], dt, kind="ExternalInput").ap()
    w = nc.dram_tensor("w_score", [A], dt, kind="ExternalInput").ap()
    ctx_out = nc.dram_tensor("context", [BL, FDIM], dt, kind="ExternalOutput").ap()
    wts_out = nc.dram_tensor("weights", [BL, S], dt, kind="ExternalOutput").ap()

    with tile.TileContext(nc) as tc, ExitStack() as ctx:
        const = ctx.enter_context(tc.tile_pool(name="const", bufs=1))

        ident = const.tile([P, P], dt)
        make_identity(nc, ident[:])

        # w_score broadcast to all partitions: w_bc[p, a] = w_score[a]
        w_bc = const.tile([P, A], dt)
        nc.sync.dma_start(w_bc[:], w[None, :].broadcast_to([P, A]))

        # proj_hidden = hs @ W.T  (contract over h: transpose both onto h-partitions)
        W_sb = const.tile([A, HDIM], dt)
        nc.sync.dma_start(W_sb[:], W[:, :])
        hs_sb = const.tile([BL, HDIM], dt)
        nc.sync.dma_start(hs_sb[:], hs[:, :])

        WT = const.tile([P, NK * A], dt)  # chunk k: WT[h, a] = W[a, k*128+h]
        hsT = const.tile([P, NK * BL], dt)  # chunk k: hsT[h, b] = hs[b, k*128+h]
        ph_sb = const.tile([BL, A], dt)
        with tc.tile_pool(name="psum_setup", bufs=2, space="PSUM") as psum:
            for k in range(NK):
                pt = psum.tile([P, A], dt, tag="tp")
                nc.tensor.transpose(pt[:], W_sb[:, k * P:(k + 1) * P], ident[:])
                nc.vector.tensor_copy(WT[:, k * A:(k + 1) * A], pt[:])
                pt2 = psum.tile([P, BL], dt, tag="tp2")
                nc.tensor.transpose(
                    pt2[:], hs_sb[:, k * P:(k + 1) * P], ident[:BL, :BL]
                )
                nc.vector.tensor_copy(hsT[:, k * BL:(k + 1) * BL], pt2[:])

            ph_psum = psum.tile([BL, A], dt, tag="ph")
            for k in range(NK):
                nc.tensor.matmul(
                    ph_psum[:],
                    lhsT=hsT[:, k * BL:(k + 1) * BL],
                    rhs=WT[:, k * A:(k + 1) * A],
                    start=(k == 0),
                    stop=(k == NK - 1),
                )
            nc.vector.tensor_copy(ph_sb[:], ph_psum[:])

        # ph_bc[b]: proj_hidden[b, :] broadcast to all 128 partitions.
        # Engines can't broadcast across partitions from an arbitrary start
        # partition, so roundtrip through DRAM and use a step-0 DMA source AP.
        ph_dram = nc.dram_tensor("ph_scratch", [BL, A], dt).ap()
        nc.sync.dma_start(ph_dram[:, :], ph_sb[:])
        ph_bc = const.tile([P, BL * A], dt)
        nc.sync.dma_start(
            ph_bc[:].rearrange("p (b a) -> p b a", a=A),
            ph_dram[None, :, :].broadcast_to([P, BL, A]),
        )

        proj_pool = ctx.enter_context(tc.tile_pool(name="proj", bufs=3))
        feat_pool = ctx.enter_context(tc.tile_pool(name="feat", bufs=2))
        sc_pool = ctx.enter_context(tc.tile_pool(name="sc", bufs=4))
        out_pool = ctx.enter_context(tc.tile_pool(name="outs", bufs=1))
        cpsum = ctx.enter_context(tc.tile_pool(name="cpsum", bufs=2, space="PSUM"))

        ctx_all = out_pool.tile([1, BL * FDIM], dt)  # context rows, concatenated

        # s is tiled as s = p*NJ + j: each partition reads ONE contiguous run
        # per DMA (16KB proj / 64KB feat) -> 128 big descriptors instead of
        # 4096 small ones, so HWDGE descriptor generation can't throttle DMA.
        for b in range(BL):
            pt = proj_pool.tile([P, S], dt, tag="proj")
            p3 = pt[:].rearrange("p (j a) -> p j a", a=A)
            nc.sync.dma_start(
                pt[:], proj[b].rearrange("s a -> (s a)").rearrange("(p x) -> p x", p=P)
            )
            # One 8 MiB DMA per batch: 64KB contiguous per partition is the
            # descriptor size that sustains line rate (smaller chunks measured
            # strictly slower end-to-end).
            ft = feat_pool.tile([P, NJ * FDIM], f32r, tag="feat")
            f3 = ft[:].rearrange("p (j d) -> p j d", d=FDIM)
            nc.sync.dma_start(
                ft[:], feat[b].rearrange("s d -> (s d)").rearrange("(p x) -> p x", p=P)
            )
            ph_b = ph_bc[:, b * A:(b + 1) * A][:, None, :].broadcast_to([P, NJ, A])
            w_b = w_bc[:][:, None, :].broadcast_to([P, NJ, A])

            # in-place: proj += ph ; proj = tanh(proj) ; proj *= w ; reduce over a
            nc.vector.tensor_add(p3, p3, ph_b)
            nc.scalar.activation(pt[:], pt[:], AF.Tanh)
            nc.vector.tensor_mul(p3, p3, w_b)
            scores = sc_pool.tile([P, NJ], dt, tag="scores")
            nc.vector.tensor_reduce(
                scores[:], p3, axis=mybir.AxisListType.X, op=mybir.AluOpType.add
            )

            # softmax over all 4096 (partitions x 32 cols), no max-subtraction
            rowsum = sc_pool.tile([P, 1], dt, tag="rowsum")
            nc.scalar.activation(scores[:], scores[:], AF.Exp, accum_out=rowsum[:])
            tot = sc_pool.tile([P, 1], dt, tag="tot")
            nc.gpsimd.partition_all_reduce(
                tot[:], rowsum[:], channels=P, reduce_op=bass_isa.ReduceOp.add
            )
            nc.vector.reciprocal(tot[:], tot[:])
            nc.vector.tensor_scalar_mul(scores[:], scores[:], tot[:])

            # weights out: with s = p*NJ + j, scores[p, j] maps straight onto
            # DRAM rows -- write directly on the idle SWDGE (gpsimd) queue.
            nc.gpsimd.dma_start(
                wts_out[b].rearrange("(p j) -> p j", p=P), scores[:]
            )

            # context: accumulate sum_j weights[:, j].T @ feat[:, j, :] on TensorE.
            # float32r streams at full rate for moving dim >= 256 (f32 is 1/4).
            scores_r = sc_pool.tile([P, NJ], f32r, tag="scores_r")
            nc.vector.tensor_copy(scores_r[:], scores[:])
            cp = cpsum.tile([1, FDIM], dt, tag="ctx")
            for j in range(NJ):
                nc.tensor.matmul(
                    cp[:],
                    lhsT=scores_r[:, j:j + 1],
                    rhs=f3[:, j, :],
                    start=(j == 0),
                    stop=(j == NJ - 1),
                )
            nc.vector.tensor_copy(ctx_all[:, b * FDIM:(b + 1) * FDIM], cp[:])

        nc.gpsimd.dma_start(ctx_out.rearrange("b d -> (b d)")[None, :], ctx_all[:])

    nc.compile()
    return nc


_NC_CACHE = None


def _get_nc():
    global _NC_CACHE
    if _NC_CACHE is None:
        _NC_CACHE = _build_nc()
    return _NC_CACHE


def kernel(proj_image, image_features, hidden_state, W_hidden, w_score):
    from concourse.bass_utils import run_bass_kernel_spmd

    proj_image = np.ascontiguousarray(np.asarray(proj_image, dtype=np.float32))
    image_features = np.ascontiguousarray(np.asarray(image_features, dtype=np.float32))
    hidden_state = np.ascontiguousarray(np.asarray(hidden_state, dtype=np.float32))
    W_hidden = np.ascontiguousarray(np.asarray(W_hidden, dtype=np.float32))
    w_score = np.ascontiguousarray(np.asarray(w_score, dtype=np.float32))

    nc = _get_nc()
    in_maps = []
    for c in range(NCORES):
        lo, hi = c * BL, (c + 1) * BL
        in_maps.append(
            {
                "proj_image": proj_image[lo:hi],
                "image_features": image_features[lo:hi],
                "hidden_state": hidden_state[lo:hi],
                "W_hidden": W_hidden,
                "w_score": w_score,
            }
        )

    res = run_bass_kernel_spmd(nc, in_maps, core_ids=list(range(NCORES)))
    context = np.concatenate([res.results[c]["context"] for c in range(NCORES)], axis=0)
    weights = np.concatenate([res.results[c]["weights"] for c in range(NCORES)], axis=0)
    return context, weights


# revision 37
# speedup vs baseline: 1.1393x; 1.0730x over previous
"""Additive attention (Bahdanau-style) Trainium2 kernel, 8-core data-parallel.

Reference computation (per batch b):
    proj_hidden[b, :]  = hidden_state[b] @ W_hidden.T                  # [A]
    combined[b, s, :]  = tanh(proj_image[b, s] + proj_hidden[b])       # [S, A]
    scores[b, s]       = combined[b, s] @ w_score                      # [S]
    weights[b, :]      = softmax(scores[b])                            # [S]
    context[b, :]      = weights[b] @ image_features[b]                # [FDIM]

Sharding: B=64 split as 8 batches per core; W_hidden / w_score replicated.

Per-core layout: S is tiled as s = p*32 + j with p on SBUF partitions, so each
partition's slice of a batch is ONE contiguous DRAM run (16 KiB proj / 64 KiB
feat) -> 128 large DMA descriptors per transfer, which is what sustains
~400 GB/s; the j*128+p layout (512 B/2 KiB descriptors) measures ~25% slower.
  - proj tile  [128(p), 32(j)*128(a)]   (2 MiB, one DMA per batch)
  - feat tile  [128(p), 32(j)*512(d)]   (8 MiB, one DMA per batch, float32r)
  - scores     [128(p), 32(j)]  -> softmax along free axis + partition_all_reduce
  - context    accumulated on TensorE: sum_j weights[:, j].T @ feat[:, j, :]
Softmax skips the max-subtraction: |scores| <= sum|w_score| ~ 9, safe in f32.
"""

import numpy as np
from contextlib import ExitStack

B, S, A, FDIM, HDIM = 64, 4096, 128, 512, 512
NCORES = 8
BL = B // NCORES  # 8 batches per core
P = 128
NJ = S // P  # 32
NK = HDIM // P  # 4


def _build_nc():
    import concourse.tile as tile
    from concourse import bacc, mybir, bass_isa
    from concourse.masks import make_identity

    dt = mybir.dt.float32
    AF = mybir.ActivationFunctionType

    nc = bacc.Bacc(
        "TRN2",
        target_bir_lowering=False,
        debug=False,
        enable_asserts=True,
        num_devices=NCORES,
    )

    f32r = mybir.dt.float32r
    proj = nc.dram_tensor("proj_image", [BL, S, A], dt, kind="ExternalInput").ap()
    # float32r: identical f32 bits on the IO path (dt.np(float32r) == float32),
    # but lets the context matmul run in fp32r mode (full-rate streaming vs 1/4
    # for plain fp32) with the BIR verifier's producer-dtype rule satisfied.
    feat = nc.dram_tensor(
        "image_features", [BL, S, FDIM], f32r, kind="ExternalInput"
    ).ap()
    hs = nc.dram_tensor("hidden_state", [BL, HDIM], dt, kind="ExternalInput").ap()
    W = nc.dram_tensor("W_hidden", [A, HDIM], dt, kind="ExternalInput").ap()
    w = nc.dram_tensor("w_score", [A], dt, kind="ExternalInput").ap()
    ctx_out = nc.dram_tensor("context", [BL, FDIM], dt, kind="ExternalOutput").ap()
    wts_out = nc.dram_tensor("weights", [BL, S], dt, kind="ExternalOutput").ap()

    with tile.TileContext(nc) as tc, ExitStack() as ctx:
        const = ctx.enter_context(tc.tile_pool(name="const", bufs=1))

        ident = const.tile([P, P], dt)
        make_identity(nc, ident[:])

        # w_score broadcast to all partitions: w_bc[p, a] = w_score[a]
        w_bc = const.tile([P, A], dt)
        nc.sync.dma_start(w_bc[:], w[None, :].broadcast_to([P, A]))

        # proj_hidden = hs @ W.T  (contract over h: transpose both onto h-partitions)
        W_sb = const.tile([A, HDIM], dt)
        nc.sync.dma_start(W_sb[:], W[:, :])
        hs_sb = const.tile([BL, HDIM], dt)
        nc.sync.dma_start(hs_sb[:], hs[:, :])

        WT = const.tile([P, NK * A], dt)  # chunk k: WT[h, a] = W[a, k*128+h]
        hsT = const.tile([P, NK * BL], dt)  # chunk k: hsT[h, b] = hs[b, k*128+h]
        ph_sb = const.tile([BL, A], dt)
        with tc.tile_pool(name="psum_setup", bufs=2, space="PSUM") as psum:
            for k in range(NK):
                pt = psum.tile([P, A], dt, tag="tp")
                nc.tensor.transpose(pt[:], W_sb[:, k * P:(k + 1) * P], ident[:])
                nc.vector.tensor_copy(WT[:, k * A:(k + 1) * A], pt[:])
                pt2 = psum.tile([P, BL], dt, tag="tp2")
                nc.tensor.transpose(
                    pt2[:], hs_sb[:, k * P:(k + 1) * P], ident[:BL, :BL]
                )
                nc.vector.tensor_copy(hsT[:, k * BL:(k + 1) * BL], pt2[:])

            ph_psum = psum.tile([BL, A], dt, tag="ph")
            for k in range(NK):
                nc.tensor.matmul(
                    ph_psum[:],
                    lhsT=hsT[:, k * BL:(k + 1) * BL],
                    rhs=WT[:, k * A:(k + 1) * A],
                    start=(k == 0),
                    stop=(k == NK - 1),
                )
            nc.vector.tensor_copy(ph_sb[:], ph_psum[:])

        # ph_bc[b]: proj_hidden[b, :] broadcast to all 128 partitions.
        # Engines can't broadcast across partitions from an arbitrary start
        # partition, so roundtrip through DRAM and use a step-0 DMA source AP.
        # On the SWDGE (gpsimd) queue: these waits must not stall the sync
        # ring, which has to start streaming proj0/feat0 immediately.
        ph_dram = nc.dram_tensor("ph_scratch", [BL, A], dt).ap()
        nc.gpsimd.dma_start(ph_dram[:, :], ph_sb[:])
        ph_bc = const.tile([P, BL * A], dt)
        nc.gpsimd.dma_start(
            ph_bc[:].rearrange("p (b a) -> p b a", a=A),
            ph_dram[None, :, :].broadcast_to([P, BL, A]),
        )

        proj_pool = ctx.enter_context(tc.tile_pool(name="proj", bufs=3))
        feat_pool = ctx.enter_context(tc.tile_pool(name="feat", bufs=2))
        sc_pool = ctx.enter_context(tc.tile_pool(name="sc", bufs=4))
        out_pool = ctx.enter_context(tc.tile_pool(name="outs", bufs=1))
        cpsum = ctx.enter_context(tc.tile_pool(name="cpsum", bufs=2, space="PSUM"))

        ctx_all = out_pool.tile([1, BL * FDIM], dt)  # context rows, concatenated

        # s is tiled as s = p*NJ + j: each partition reads ONE contiguous run
        # per DMA (16KB proj / 64KB feat) -> 128 big descriptors instead of
        # 4096 small ones, so HWDGE descriptor generation can't throttle DMA.
        for b in range(BL):
            pt = proj_pool.tile([P, S], dt, tag="proj")
            p3 = pt[:].rearrange("p (j a) -> p j a", a=A)
            nc.sync.dma_start(
                pt[:], proj[b].rearrange("s a -> (s a)").rearrange("(p x) -> p x", p=P)
            )
            # One 8 MiB DMA per batch: 64KB contiguous per partition is the
            # descriptor size that sustains line rate (smaller chunks measured
            # strictly slower end-to-end).
            ft = feat_pool.tile([P, NJ * FDIM], f32r, tag="feat")
            f3 = ft[:].rearrange("p (j d) -> p j d", d=FDIM)
            nc.sync.dma_start(
                ft[:], feat[b].rearrange("s d -> (s d)").rearrange("(p x) -> p x", p=P)
            )
            ph_b = ph_bc[:, b * A:(b + 1) * A][:, None, :].broadcast_to([P, NJ, A])
            w_b = w_bc[:][:, None, :].broadcast_to([P, NJ, A])

            # in-place: proj += ph ; proj = tanh(proj) ; proj *= w ; reduce over a
            nc.vector.tensor_add(p3, p3, ph_b)
            nc.scalar.activation(pt[:], pt[:], AF.Tanh)
            nc.vector.tensor_mul(p3, p3, w_b)
            scores = sc_pool.tile([P, NJ], dt, tag="scores")
            nc.vector.tensor_reduce(
                scores[:], p3, axis=mybir.AxisListType.X, op=mybir.AluOpType.add
            )

            # softmax over all 4096 (partitions x 32 cols), no max-subtraction
            rowsum = sc_pool.tile([P, 1], dt, tag="rowsum")
            nc.scalar.activation(scores[:], scores[:], AF.Exp, accum_out=rowsum[:])
            tot = sc_pool.tile([P, 1], dt, tag="tot")
            nc.gpsimd.partition_all_reduce(
                tot[:], rowsum[:], channels=P, reduce_op=bass_isa.ReduceOp.add
            )
            nc.vector.reciprocal(tot[:], tot[:])
            nc.vector.tensor_scalar_mul(scores[:], scores[:], tot[:])

            # weights out: with s = p*NJ + j, scores[p, j] maps straight onto
            # DRAM rows -- write directly on the idle SWDGE (gpsimd) queue.
            nc.gpsimd.dma_start(
                wts_out[b].rearrange("(p j) -> p j", p=P), scores[:]
            )

            # context: accumulate sum_j weights[:, j].T @ feat[:, j, :] on TensorE.
            # float32r streams at full rate for moving dim >= 256 (f32 is 1/4).
            scores_r = sc_pool.tile([P, NJ], f32r, tag="scores_r")
            nc.vector.tensor_copy(scores_r[:], scores[:])
            cp = cpsum.tile([1, FDIM], dt, tag="ctx")
            for j in range(NJ):
                nc.tensor.matmul(
                    cp[:],
                    lhsT=scores_r[:, j:j + 1],
                    rhs=f3[:, j, :],
                    start=(j == 0),
                    stop=(j == NJ - 1),
                )
            nc.vector.tensor_copy(ctx_all[:, b * FDIM:(b + 1) * FDIM], cp[:])
            # write each context row out as soon as it exists (idle SWDGE
            # queue) so only batch 7's 2KB write remains after the last matmul
            nc.gpsimd.dma_start(
                ctx_out[b][None, :], ctx_all[:, b * FDIM:(b + 1) * FDIM]
            )

    nc.compile()
    return nc


_NC_CACHE = None


def _get_nc():
    global _NC_CACHE
    if _NC_CACHE is None:
        _NC_CACHE = _build_nc()
    return _NC_CACHE


def kernel(proj_image, image_features, hidden_state, W_hidden, w_score):
    from concourse.bass_utils import run_bass_kernel_spmd

    proj_image = np.ascontiguousarray(np.asarray(proj_image, dtype=np.float32))
    image_features = np.ascontiguousarray(np.asarray(image_features, dtype=np.float32))
    hidden_state = np.ascontiguousarray(np.asarray(hidden_state, dtype=np.float32))
    W_hidden = np.ascontiguousarray(np.asarray(W_hidden, dtype=np.float32))
    w_score = np.ascontiguousarray(np.asarray(w_score, dtype=np.float32))

    nc = _get_nc()
    in_maps = []
    for c in range(NCORES):
        lo, hi = c * BL, (c + 1) * BL
        in_maps.append(
            {
                "proj_image": proj_image[lo:hi],
                "image_features": image_features[lo:hi],
                "hidden_state": hidden_state[lo:hi],
                "W_hidden": W_hidden,
                "w_score": w_score,
            }
        )

    res = run_bass_kernel_spmd(nc, in_maps, core_ids=list(range(NCORES)))
    context = np.concatenate([res.results[c]["context"] for c in range(NCORES)], axis=0)
    weights = np.concatenate([res.results[c]["weights"] for c in range(NCORES)], axis=0)
    return context, weights
